# revision 8
# baseline (speedup 1.0000x reference)
"""Trainium2 Bass kernel for nn_MatchingPursuit (ADMM LASSO, 50 iters).

Math (per batch row b of x):
  wu = weight / ||weight||_row                (4096, 1024)
  A = wu.T ; M_inv = inv(A^T A + I_4096)
  Woodbury: p @ M_inv = p - (p @ U1) @ V2,  U1 = wu @ inv(I + wu.T wu),
                                            V2 = wu.T
  ADMM recurrence rewritten in q-form:
    q_1 = Atb - r_1,            r_t = (p_t @ U1) @ V2
    p_1 = Atb
    for t = 2..50:
      v_{t-1} = soft(q_{t-1}, 0.2)
      p_t = Atb + 2 v_{t-1} - q_{t-1}
      q_t = (Atb + v_{t-1}) - r_t
    z = soft(q_50);  decoded = z @ wu

Distribution: pure data-parallel over batch. 512 rows -> 8 cores x 64.
Weights (U1, V2 fp16) replicated and resident in SBUF; fp32 state.

On-chip layout ("folded" batch-major): state tensors are (128, 2048) where
partition = b + 64*(n // 2048), free = n % 2048 for feature n in [0,4096).
Matmuls put weights on the moving operand (N=512) and small transposed
activation tiles (128, 64) fp16 on the stationary side; the two column
halves of the PE array run two concurrent chains (tile_position (0,0) /
(0,64)) writing psum partitions 0:64 / 64:128, which reproduces the folded
layout for free.
"""

import numpy as np

LAMBD = 0.2
ITERS = 50
B = 512
M = 1024  # in_features
N = 4096  # out_features
NCORES = 8
BL = B // NCORES  # 64 rows per core

_PROGRAM_CACHE = {}


def _build_program(iters=ITERS):
    import concourse.bass as bass
    import concourse.bacc as bacc
    import concourse.tile as tile
    import concourse.mybir as mybir

    f32 = mybir.dt.float32
    f16 = mybir.dt.float16
    AF = mybir.ActivationFunctionType
    OP = mybir.AluOpType

    nc = bacc.Bacc(None)

    # ---- DRAM parameters ----
    xT16_d = nc.declare_dram_parameter("xT16", [8, 128, BL], f16, isOutput=False)
    u1_d = nc.declare_dram_parameter("u1", [32, 128, M], f16, isOutput=False)
    v2_d = nc.declare_dram_parameter("v2", [8, 128, N], f16, isOutput=False)
    wu_d = nc.declare_dram_parameter("wu16", [32, 128, M], f16, isOutput=False)
    id_d = nc.declare_dram_parameter("ident", [128, 128], f32, isOutput=False)
    z_d = nc.declare_dram_parameter("z_out", [BL, N], f32, isOutput=True)
    dec_d = nc.declare_dram_parameter("dec_out", [BL, M], f32, isOutput=True)

    with tile.TileContext(nc) as tc:
        with (
            tc.tile_pool(name="consts", bufs=1) as consts,
            tc.tile_pool(name="state", bufs=1) as state,
            tc.tile_pool(name="small", bufs=1) as small,
            tc.tile_pool(name="wustage", bufs=2) as wustage,
            tc.tile_pool(name="ps_s", bufs=1, space="PSUM") as ps_s_pool,
            tc.tile_pool(name="ps_r", bufs=4, space="PSUM") as ps_r_pool,
            tc.tile_pool(name="ps_tr", bufs=2, space="PSUM") as ps_tr_pool,
        ):
            u1_sb = consts.tile([128, 32 * M], f16)     # 8 MB: U1 tiles k
            v2_sb = consts.tile([128, 8 * N], f16)      # 8 MB: V2 tiles j
            ident = consts.tile([128, 128], f32)
            xT = consts.tile([128, 8 * BL], f16)        # xT tiles j

            atb = state.tile([128, 2048], f32)
            q = state.tile([128, 2048], f32)
            v = state.tile([128, 2048], f32)
            t = state.tile([128, 2048], f32)
            cb = state.tile([128, 2048], f32)

            pT = small.tile([128, 16 * 128], f16)       # transposed act tiles
            sT = small.tile([128, 8 * BL], f16)         # transposed s tiles
            s32 = small.tile([128, 512], f32)
            neg_lam = small.tile([128, 1], f32)
            nc.gpsimd.memset(neg_lam[:], -LAMBD)

            # ---- load constants ----
            nc.sync.dma_start(out=ident[:], in_=id_d[:])
            for j in range(8):
                nc.sync.dma_start(out=xT[:, j * BL:(j + 1) * BL], in_=xT16_d[j])
            for j in range(8):
                nc.sync.dma_start(out=v2_sb[:, j * N:(j + 1) * N], in_=v2_d[j])
            for k in range(32):
                nc.sync.dma_start(out=u1_sb[:, k * M:(k + 1) * M], in_=u1_d[k])

            def mm2_like(lhsT_of_j, out_banks):
                """r[b,n] = sum_m s[b,m] V2[m,n] -> folded psum banks."""
                for j in range(8):
                    lhsT = lhsT_of_j(j)
                    for nb in range(4):
                        base = j * N + 512 * nb
                        nc.tensor.matmul(
                            out_banks[nb][0:64, :], lhsT,
                            v2_sb[:, base:base + 512],
                            start=(j == 0), stop=(j == 7),
                            tile_position=(0, 0), skip_group_check=True)
                        nc.tensor.matmul(
                            out_banks[nb][64:128, :], lhsT,
                            v2_sb[:, base + 2048:base + 2560],
                            start=(j == 0), stop=(j == 7),
                            tile_position=(0, 64), skip_group_check=True)

            def transpose_in(src, dst16):
                """src (128,2048) f32 folded -> dst16 (128, 16*128) f16 tiles."""
                for i in range(16):
                    tr = ps_tr_pool.tile([128, 128], f32, tag="tr")
                    nc.tensor.transpose(
                        tr[:], src[:, 128 * i:128 * (i + 1)], ident[:])
                    nc.scalar.copy(dst16[:, 128 * i:128 * (i + 1)], tr[:])

            def mm1(out_ps):
                """s[b,m] = sum_n t[b,n] U1[n,m] -> folded psum (128,512)."""
                for k in range(32):
                    i = k % 16
                    half = 64 * (k // 16)
                    lhsT = pT[:, 128 * i + half:128 * i + half + 64]
                    nc.tensor.matmul(
                        out_ps[0:64, :], lhsT,
                        u1_sb[:, M * k:M * k + 512],
                        start=(k == 0), stop=(k == 31), tile_position=(0, 0), skip_group_check=True)
                    nc.tensor.matmul(
                        out_ps[64:128, :], lhsT,
                        u1_sb[:, M * k + 512:M * k + 1024],
                        start=(k == 0), stop=(k == 31), tile_position=(0, 64), skip_group_check=True)

            def s_transposes(src32, dst16):
                """src32 (128,512) f32 folded s -> dst16 (128, 8*64) f16."""
                for j in range(8):
                    jj = j % 4
                    half = 64 * (j // 4)
                    tr = ps_tr_pool.tile([128, 64], f32, tag="tr")
                    nc.tensor.transpose(
                        tr[:], src32[half:half + 64, 128 * jj:128 * (jj + 1)],
                        ident[half:half + 64, half:half + 64])
                    nc.scalar.copy(dst16[:, BL * j:BL * (j + 1)], tr[:])

            # ---- prologue: Atb = x @ V2 (stationary = xT tiles) ----
            ps_atb = [ps_r_pool.tile([128, 512], f32, tag="r", name=f"ps_atb{i}") for i in range(4)]
            for j in range(8):
                lhsT = xT[:, BL * j:BL * (j + 1)]
                for nb in range(4):
                    base = j * N + 512 * nb
                    nc.tensor.matmul(ps_atb[nb][0:64, :], lhsT,
                                     v2_sb[:, base:base + 512],
                                     start=(j == 0), stop=(j == 7),
                                     tile_position=(0, 0), skip_group_check=True)
                    nc.tensor.matmul(ps_atb[nb][64:128, :], lhsT,
                                     v2_sb[:, base + 2048:base + 2560],
                                     start=(j == 0), stop=(j == 7),
                                     tile_position=(0, 64), skip_group_check=True)
            for nb in range(4):
                nc.vector.tensor_copy(atb[:, 512 * nb:512 * (nb + 1)],
                                      ps_atb[nb][:])

            # ---- iteration 1: p_1 = Atb, q_1 = Atb - r_1 ----
            transpose_in(atb, pT)
            ps_s = ps_s_pool.tile([128, 512], f32, tag="s")
            mm1(ps_s)
            nc.vector.tensor_copy(s32[:], ps_s[:])
            s_transposes(s32, sT)
            ps_r = [ps_r_pool.tile([128, 512], f32, tag="r", name=f"ps_r{i}")
                    for i in range(4)]
            mm2_like(lambda j: sT[:, BL * j:BL * (j + 1)], ps_r)
            for nb in range(4):
                nc.vector.tensor_tensor(q[:, 512 * nb:512 * (nb + 1)],
                                        atb[:, 512 * nb:512 * (nb + 1)],
                                        ps_r[nb][:], op=OP.subtract)

            # ---- iterations 2..iters ----
            for _ in range(iters - 1):
                # v = soft(q) = relu(q - l) + min(q + l, 0)
                nc.scalar.activation(v[:], q[:], AF.Relu, bias=neg_lam[:], scale=1.0)
                nc.gpsimd.tensor_scalar(cb[:], q[:], LAMBD, 0.0,
                                        op0=OP.add, op1=OP.min)
                nc.vector.tensor_tensor(v[:], v[:], cb[:], op=OP.add)
                # t = Atb + 2v - q
                nc.vector.scalar_tensor_tensor(t[:], v[:], 2.0, q[:],
                                               op0=OP.mult, op1=OP.subtract)
                nc.vector.tensor_tensor(t[:], t[:], atb[:], op=OP.add)
                # cb = Atb + v   (consumed after mm2)
                nc.vector.tensor_tensor(cb[:], atb[:], v[:], op=OP.add)

                transpose_in(t, pT)
                ps_s = ps_s_pool.tile([128, 512], f32, tag="s")
                mm1(ps_s)
                nc.vector.tensor_copy(s32[:], ps_s[:])
                s_transposes(s32, sT)
                ps_r = [ps_r_pool.tile([128, 512], f32, tag="r", name=f"ps_ri{i}")
                        for i in range(4)]
                mm2_like(lambda j: sT[:, BL * j:BL * (j + 1)], ps_r)
                for nb in range(4):
                    nc.vector.tensor_tensor(q[:, 512 * nb:512 * (nb + 1)],
                                            cb[:, 512 * nb:512 * (nb + 1)],
                                            ps_r[nb][:], op=OP.subtract)

            # ---- epilogue: z = soft(q), decoded = z @ wu ----
            nc.scalar.activation(v[:], q[:], AF.Relu, bias=neg_lam[:], scale=1.0)
            nc.gpsimd.tensor_scalar(cb[:], q[:], LAMBD, 0.0,
                                    op0=OP.add, op1=OP.min)
            nc.vector.tensor_tensor(v[:], v[:], cb[:], op=OP.add)
            nc.sync.dma_start(out=z_d[:, 0:2048], in_=v[0:64, :])
            nc.sync.dma_start(out=z_d[:, 2048:4096], in_=v[64:128, :])

            # decoded: stationary zT tiles, moving wu (streamed from HBM)
            transpose_in(v, pT)
            ps_dec = ps_s_pool.tile([128, 512], f32, tag="s")
            for k in range(32):
                i = k % 16
                half = 64 * (k // 16)
                lhsT = pT[:, 128 * i + half:128 * i + half + 64]
                stage = wustage.tile([128, M], f16, tag="wu")
                nc.sync.dma_start(out=stage[:], in_=wu_d[k])
                nc.tensor.matmul(ps_dec[0:64, :], lhsT, stage[:, 0:512],
                                 start=(k == 0), stop=(k == 31),
                                 tile_position=(0, 0), skip_group_check=True)
                nc.tensor.matmul(ps_dec[64:128, :], lhsT, stage[:, 512:1024],
                                 start=(k == 0), stop=(k == 31),
                                 tile_position=(0, 64), skip_group_check=True)
            dec32 = small.tile([128, 512], f32)
            nc.vector.tensor_copy(dec32[:], ps_dec[:])
            nc.sync.dma_start(out=dec_d[:, 0:512], in_=dec32[0:64, :])
            nc.sync.dma_start(out=dec_d[:, 512:1024], in_=dec32[64:128, :])

    nc.compile()
    return nc


def _host_prep(x, weight):
    """Host-side weight preparation (float64 for the small inversion)."""
    x_flat = np.asarray(x, dtype=np.float32).reshape(B, -1)
    w = np.asarray(weight, dtype=np.float64)
    wu = w / np.linalg.norm(w, axis=1, keepdims=True)        # (4096, 1024)
    G = np.eye(M) + wu.T @ wu
    Gi = np.linalg.inv(G)
    U1 = (wu @ Gi).astype(np.float32)                        # (4096, 1024)
    V2 = np.ascontiguousarray(wu.T).astype(np.float32)       # (1024, 4096)

    u1_16 = U1.astype(np.float16).reshape(32, 128, M)
    v2_16 = V2.astype(np.float16).reshape(8, 128, N)
    wu_16 = wu.astype(np.float16).reshape(32, 128, M)
    ident = np.eye(128, dtype=np.float32)
    return x_flat, u1_16, v2_16, wu_16, ident


def kernel(x, weight):
    from concourse.bass_utils import run_bass_kernel_spmd

    x_flat, u1_16, v2_16, wu_16, ident = _host_prep(x, weight)

    if "nc" not in _PROGRAM_CACHE:
        _PROGRAM_CACHE["nc"] = _build_program(ITERS)
    nc = _PROGRAM_CACHE["nc"]

    in_maps = []
    for c in range(NCORES):
        rows = x_flat[c * BL:(c + 1) * BL]                   # (64, 1024)
        xT16 = np.ascontiguousarray(rows.T).astype(np.float16).reshape(
            8, 128, BL)
        in_maps.append({
            "xT16": xT16,
            "u1": u1_16,
            "v2": v2_16,
            "wu16": wu_16,
            "ident": ident,
        })

    res = run_bass_kernel_spmd(nc, in_maps, list(range(NCORES)))
    z = np.concatenate([res.results[c]["z_out"] for c in range(NCORES)], axis=0)
    dec = np.concatenate([res.results[c]["dec_out"] for c in range(NCORES)],
                         axis=0)
    input_shape = np.asarray(x).shape
    return z.astype(np.float32), dec.astype(np.float32).reshape(input_shape)


# revision 11
# speedup vs baseline: 1.9910x; 1.9910x over previous
"""Trainium2 Bass kernel for nn_MatchingPursuit (ADMM LASSO, 50 iters).

Math (per batch row b of x):
  wu = weight / ||weight||_row                (4096, 1024)
  A = wu.T ; M_inv = inv(A^T A + I_4096)
  Woodbury: p @ M_inv = p - (p @ U1) @ V2,  U1 = wu @ inv(I + wu.T wu),
                                            V2 = wu.T
  ADMM recurrence rewritten in q-form:
    q_1 = Atb - r_1,            r_t = (p_t @ U1) @ V2
    p_1 = Atb
    for t = 2..50:
      v_{t-1} = soft(q_{t-1}, 0.2)
      p_t = Atb + 2 v_{t-1} - q_{t-1}
      q_t = (Atb + v_{t-1}) - r_t
    z = soft(q_50);  decoded = z @ wu

Distribution: pure data-parallel over batch. 512 rows -> 8 cores x 64.
Weights (U1, V2 fp16) replicated and resident in SBUF; fp32 state.

On-chip layout ("folded" batch-major): state tensors are (128, 2048) where
partition = b + 64*(n // 2048), free = n % 2048 for feature n in [0,4096).
Matmuls put weights on the moving operand (N=512) and small transposed
activation tiles (128, 64) fp16 on the stationary side; the two column
halves of the PE array run two concurrent chains (tile_position (0,0) /
(0,64)) writing psum partitions 0:64 / 64:128, which reproduces the folded
layout for free.
"""

import numpy as np

LAMBD = 0.2
ITERS = 50
B = 512
M = 1024  # in_features
N = 4096  # out_features
NCORES = 8
BL = B // NCORES  # 64 rows per core

_PROGRAM_CACHE = {}


def _build_program(iters=ITERS):
    import concourse.bass as bass
    import concourse.bacc as bacc
    import concourse.tile as tile
    import concourse.mybir as mybir

    f32 = mybir.dt.float32
    f16 = mybir.dt.float16
    AF = mybir.ActivationFunctionType
    OP = mybir.AluOpType

    nc = bacc.Bacc(None)

    # ---- DRAM parameters ----
    xT16_d = nc.declare_dram_parameter("xT16", [8, 128, BL], f16, isOutput=False)
    u1_d = nc.declare_dram_parameter("u1", [32, 128, M], f16, isOutput=False)
    v2_d = nc.declare_dram_parameter("v2", [8, 128, N], f16, isOutput=False)
    wu_d = nc.declare_dram_parameter("wu16", [32, 128, M], f16, isOutput=False)
    id_d = nc.declare_dram_parameter("ident", [128, 128], f32, isOutput=False)
    z_d = nc.declare_dram_parameter("z_out", [BL, N], f32, isOutput=True)
    dec_d = nc.declare_dram_parameter("dec_out", [BL, M], f32, isOutput=True)

    with tile.TileContext(nc) as tc:
        with (
            tc.tile_pool(name="consts", bufs=1) as consts,
            tc.tile_pool(name="state", bufs=1) as state,
            tc.tile_pool(name="small", bufs=1) as small,
            tc.tile_pool(name="wustage", bufs=2) as wustage,
            tc.tile_pool(name="ps_s", bufs=1, space="PSUM") as ps_s_pool,
            tc.tile_pool(name="ps_r", bufs=4, space="PSUM") as ps_r_pool,
            tc.tile_pool(name="ps_tr", bufs=2, space="PSUM") as ps_tr_pool,
        ):
            u1_sb = consts.tile([128, 32 * M], f16)     # 8 MB: U1 tiles k
            v2_sb = consts.tile([128, 8 * N], f16)      # 8 MB: V2 tiles j
            ident = consts.tile([128, 128], f32)
            xT = consts.tile([128, 8 * BL], f16)        # xT tiles j

            atb = state.tile([128, 2048], f32)
            q = state.tile([128, 2048], f32)
            t = state.tile([128, 2048], f32)
            cb = state.tile([128, 2048], f32)
            gb = state.tile([128, 2048], f32)
            cl = state.tile([128, 2048], f32)

            pT = small.tile([128, 16 * 128], f16)       # transposed act tiles
            sT = small.tile([128, 8 * BL], f16)         # transposed s tiles
            s32 = small.tile([128, 512], f32)

            # ---- load constants ----
            nc.sync.dma_start(out=ident[:], in_=id_d[:])
            for j in range(8):
                nc.sync.dma_start(out=xT[:, j * BL:(j + 1) * BL], in_=xT16_d[j])
            for j in range(8):
                nc.sync.dma_start(out=v2_sb[:, j * N:(j + 1) * N], in_=v2_d[j])
            for k in range(32):
                nc.sync.dma_start(out=u1_sb[:, k * M:(k + 1) * M], in_=u1_d[k])

            def mm2_like(lhsT_of_j, out_banks):
                """r[b,n] = sum_m s[b,m] V2[m,n] -> folded psum banks."""
                for j in range(8):
                    lhsT = lhsT_of_j(j)
                    for nb in range(4):
                        base = j * N + 512 * nb
                        nc.tensor.matmul(
                            out_banks[nb][0:64, :], lhsT,
                            v2_sb[:, base:base + 512],
                            start=(j == 0), stop=(j == 7),
                            tile_position=(0, 0), skip_group_check=True)
                        nc.tensor.matmul(
                            out_banks[nb][64:128, :], lhsT,
                            v2_sb[:, base + 2048:base + 2560],
                            start=(j == 0), stop=(j == 7),
                            tile_position=(0, 64), skip_group_check=True)

            def transpose_in(src, dst16):
                """src (128,2048) f32 folded -> dst16 (128, 16*128) f16 tiles."""
                for i in range(16):
                    tr = ps_tr_pool.tile([128, 128], f32, tag="tr")
                    nc.tensor.transpose(
                        tr[:], src[:, 128 * i:128 * (i + 1)], ident[:])
                    nc.scalar.copy(dst16[:, 128 * i:128 * (i + 1)], tr[:])

            def mm1(out_ps):
                """s[b,m] = sum_n t[b,n] U1[n,m] -> folded psum (128,512)."""
                for k in range(32):
                    i = k % 16
                    half = 64 * (k // 16)
                    lhsT = pT[:, 128 * i + half:128 * i + half + 64]
                    nc.tensor.matmul(
                        out_ps[0:64, :], lhsT,
                        u1_sb[:, M * k:M * k + 512],
                        start=(k == 0), stop=(k == 31), tile_position=(0, 0), skip_group_check=True)
                    nc.tensor.matmul(
                        out_ps[64:128, :], lhsT,
                        u1_sb[:, M * k + 512:M * k + 1024],
                        start=(k == 0), stop=(k == 31), tile_position=(0, 64), skip_group_check=True)

            def s_transposes(src32, dst16):
                """src32 (128,512) f32 folded s -> dst16 (128, 8*64) f16."""
                for j in range(8):
                    jj = j % 4
                    half = 64 * (j // 4)
                    tr = ps_tr_pool.tile([128, 64], f32, tag="tr")
                    nc.tensor.transpose(
                        tr[:], src32[half:half + 64, 128 * jj:128 * (jj + 1)],
                        ident[half:half + 64, half:half + 64])
                    nc.scalar.copy(dst16[:, BL * j:BL * (j + 1)], tr[:])

            # ---- prologue: Atb = x @ V2 (stationary = xT tiles) ----
            ps_atb = [ps_r_pool.tile([128, 512], f32, tag="r", name=f"ps_atb{i}") for i in range(4)]
            for j in range(8):
                lhsT = xT[:, BL * j:BL * (j + 1)]
                for nb in range(4):
                    base = j * N + 512 * nb
                    nc.tensor.matmul(ps_atb[nb][0:64, :], lhsT,
                                     v2_sb[:, base:base + 512],
                                     start=(j == 0), stop=(j == 7),
                                     tile_position=(0, 0), skip_group_check=True)
                    nc.tensor.matmul(ps_atb[nb][64:128, :], lhsT,
                                     v2_sb[:, base + 2048:base + 2560],
                                     start=(j == 0), stop=(j == 7),
                                     tile_position=(0, 64), skip_group_check=True)
            for nb in range(4):
                nc.vector.tensor_copy(atb[:, 512 * nb:512 * (nb + 1)],
                                      ps_atb[nb][:])

            # ---- iteration 1: p_1 = Atb, q_1 = Atb - r_1 ----
            transpose_in(atb, pT)
            ps_s = ps_s_pool.tile([128, 512], f32, tag="s")
            mm1(ps_s)
            nc.vector.tensor_copy(s32[:], ps_s[:])
            s_transposes(s32, sT)
            ps_r = [ps_r_pool.tile([128, 512], f32, tag="r", name=f"ps_r{i}")
                    for i in range(4)]
            mm2_like(lambda j: sT[:, BL * j:BL * (j + 1)], ps_r)
            for nb in range(4):
                nc.vector.tensor_tensor(q[:, 512 * nb:512 * (nb + 1)],
                                        atb[:, 512 * nb:512 * (nb + 1)],
                                        ps_r[nb][:], op=OP.subtract)

            # ---- iterations 2..iters ----
            # Elementwise in clamp form, chunked by psum bank for
            # pipelining:  clamp = min(max(q,-l),l); g = Atb + q;
            # t = g - 2*clamp (= Atb + 2*soft(q) - q);  c = g - clamp.
            for _ in range(iters - 1):
                for nb in range(4):
                    sl = slice(512 * nb, 512 * (nb + 1))
                    nc.vector.tensor_scalar(cl[:, sl], q[:, sl],
                                            -LAMBD, LAMBD,
                                            op0=OP.max, op1=OP.min)
                    nc.vector.tensor_tensor(gb[:, sl], atb[:, sl], q[:, sl],
                                            op=OP.add)
                    nc.vector.scalar_tensor_tensor(t[:, sl], cl[:, sl], -2.0,
                                                   gb[:, sl],
                                                   op0=OP.mult, op1=OP.add)
                    for i in range(4 * nb, 4 * nb + 4):
                        tr = ps_tr_pool.tile([128, 128], f32, tag="tr")
                        nc.tensor.transpose(
                            tr[:], t[:, 128 * i:128 * (i + 1)], ident[:])
                        nc.scalar.copy(pT[:, 128 * i:128 * (i + 1)], tr[:])
                for nb in range(4):
                    sl = slice(512 * nb, 512 * (nb + 1))
                    nc.vector.tensor_tensor(cb[:, sl], gb[:, sl], cl[:, sl],
                                            op=OP.subtract)
                ps_s = ps_s_pool.tile([128, 512], f32, tag="s")
                mm1(ps_s)
                nc.scalar.copy(s32[:], ps_s[:])
                s_transposes(s32, sT)
                ps_r = [ps_r_pool.tile([128, 512], f32, tag="r", name=f"ps_ri{i}")
                        for i in range(4)]
                mm2_like(lambda j: sT[:, BL * j:BL * (j + 1)], ps_r)
                for nb in range(4):
                    nc.vector.tensor_tensor(q[:, 512 * nb:512 * (nb + 1)],
                                            cb[:, 512 * nb:512 * (nb + 1)],
                                            ps_r[nb][:], op=OP.subtract)

            # ---- epilogue: z = soft(q) = q - clamp(q), decoded = z @ wu ----
            nc.vector.tensor_scalar(cl[:], q[:], -LAMBD, LAMBD,
                                    op0=OP.max, op1=OP.min)
            nc.vector.tensor_tensor(t[:], q[:], cl[:], op=OP.subtract)
            nc.sync.dma_start(out=z_d[:, 0:2048], in_=t[0:64, :])
            nc.sync.dma_start(out=z_d[:, 2048:4096], in_=t[64:128, :])

            # decoded: stationary zT tiles, moving wu (streamed from HBM)
            transpose_in(t, pT)
            ps_dec = ps_s_pool.tile([128, 512], f32, tag="s")
            for k in range(32):
                i = k % 16
                half = 64 * (k // 16)
                lhsT = pT[:, 128 * i + half:128 * i + half + 64]
                stage = wustage.tile([128, M], f16, tag="wu")
                nc.sync.dma_start(out=stage[:], in_=wu_d[k])
                nc.tensor.matmul(ps_dec[0:64, :], lhsT, stage[:, 0:512],
                                 start=(k == 0), stop=(k == 31),
                                 tile_position=(0, 0), skip_group_check=True)
                nc.tensor.matmul(ps_dec[64:128, :], lhsT, stage[:, 512:1024],
                                 start=(k == 0), stop=(k == 31),
                                 tile_position=(0, 64), skip_group_check=True)
            dec32 = small.tile([128, 512], f32)
            nc.vector.tensor_copy(dec32[:], ps_dec[:])
            nc.sync.dma_start(out=dec_d[:, 0:512], in_=dec32[0:64, :])
            nc.sync.dma_start(out=dec_d[:, 512:1024], in_=dec32[64:128, :])

    nc.compile()
    return nc


def _host_prep(x, weight):
    """Host-side weight preparation (float64 for the small inversion)."""
    x_flat = np.asarray(x, dtype=np.float32).reshape(B, -1)
    w = np.asarray(weight, dtype=np.float64)
    wu = w / np.linalg.norm(w, axis=1, keepdims=True)        # (4096, 1024)
    G = np.eye(M) + wu.T @ wu
    Gi = np.linalg.inv(G)
    U1 = (wu @ Gi).astype(np.float32)                        # (4096, 1024)
    V2 = np.ascontiguousarray(wu.T).astype(np.float32)       # (1024, 4096)

    u1_16 = U1.astype(np.float16).reshape(32, 128, M)
    v2_16 = V2.astype(np.float16).reshape(8, 128, N)
    wu_16 = wu.astype(np.float16).reshape(32, 128, M)
    ident = np.eye(128, dtype=np.float32)
    return x_flat, u1_16, v2_16, wu_16, ident


def kernel(x, weight):
    from concourse.bass_utils import run_bass_kernel_spmd

    x_flat, u1_16, v2_16, wu_16, ident = _host_prep(x, weight)

    if "nc" not in _PROGRAM_CACHE:
        _PROGRAM_CACHE["nc"] = _build_program(ITERS)
    nc = _PROGRAM_CACHE["nc"]

    in_maps = []
    for c in range(NCORES):
        rows = x_flat[c * BL:(c + 1) * BL]                   # (64, 1024)
        xT16 = np.ascontiguousarray(rows.T).astype(np.float16).reshape(
            8, 128, BL)
        in_maps.append({
            "xT16": xT16,
            "u1": u1_16,
            "v2": v2_16,
            "wu16": wu_16,
            "ident": ident,
        })

    res = run_bass_kernel_spmd(nc, in_maps, list(range(NCORES)))
    z = np.concatenate([res.results[c]["z_out"] for c in range(NCORES)], axis=0)
    dec = np.concatenate([res.results[c]["dec_out"] for c in range(NCORES)],
                         axis=0)
    input_shape = np.asarray(x).shape
    return z.astype(np.float32), dec.astype(np.float32).reshape(input_shape)


# revision 14
# speedup vs baseline: 2.3318x; 1.1712x over previous
"""Trainium2 Bass kernel for nn_MatchingPursuit (ADMM LASSO, 50 iters).

Math (per batch row b of x):
  wu = weight / ||weight||_row                (4096, 1024)
  A = wu.T ; M_inv = inv(A^T A + I_4096)
  Woodbury: p @ M_inv = p - (p @ U1) @ V2,  U1 = wu @ inv(I + wu.T wu),
                                            V2 = wu.T
  ADMM recurrence rewritten in q-form:
    q_1 = Atb - r_1,            r_t = (p_t @ U1) @ V2
    p_1 = Atb
    for t = 2..50:
      v_{t-1} = soft(q_{t-1}, 0.2)
      p_t = Atb + 2 v_{t-1} - q_{t-1}
      q_t = (Atb + v_{t-1}) - r_t
    z = soft(q_50);  decoded = z @ wu

Distribution: pure data-parallel over batch. 512 rows -> 8 cores x 64.
Weights (U1, V2 fp16) replicated and resident in SBUF; fp32 state.

On-chip layout ("folded" batch-major): state tensors are (128, 2048) where
partition = b + 64*(n // 2048), free = n % 2048 for feature n in [0,4096).
Matmuls put weights on the moving operand (N=512) and small transposed
activation tiles (128, 64) fp16 on the stationary side; the two column
halves of the PE array run two concurrent chains (tile_position (0,0) /
(0,64)) writing psum partitions 0:64 / 64:128, which reproduces the folded
layout for free.
"""

import numpy as np

LAMBD = 0.2
ITERS = 50
B = 512
M = 1024  # in_features
N = 4096  # out_features
NCORES = 8
BL = B // NCORES  # 64 rows per core

_PROGRAM_CACHE = {}


def _build_program(iters=ITERS):
    import concourse.bass as bass
    import concourse.bacc as bacc
    import concourse.tile as tile
    import concourse.mybir as mybir

    f32 = mybir.dt.float32
    f16 = mybir.dt.float16
    AF = mybir.ActivationFunctionType
    OP = mybir.AluOpType

    nc = bacc.Bacc(None)

    # ---- DRAM parameters ----
    xT16_d = nc.declare_dram_parameter("xT16", [8, 128, BL], f16, isOutput=False)
    u1_d = nc.declare_dram_parameter("u1", [32, 128, M], f16, isOutput=False)
    v2_d = nc.declare_dram_parameter("v2", [8, 128, N], f16, isOutput=False)
    wu_d = nc.declare_dram_parameter("wu16", [32, 128, M], f16, isOutput=False)
    id_d = nc.declare_dram_parameter("ident", [128, 128], f32, isOutput=False)
    z_d = nc.declare_dram_parameter("z_out", [BL, N], f32, isOutput=True)
    dec_d = nc.declare_dram_parameter("dec_out", [BL, M], f32, isOutput=True)

    with tile.TileContext(nc) as tc:
        with (
            tc.tile_pool(name="consts", bufs=1) as consts,
            tc.tile_pool(name="state", bufs=1) as state,
            tc.tile_pool(name="small", bufs=1) as small,
            tc.tile_pool(name="wustage", bufs=2) as wustage,
            tc.tile_pool(name="ps_s", bufs=2, space="PSUM") as ps_s_pool,
            tc.tile_pool(name="ps_r", bufs=4, space="PSUM") as ps_r_pool,
            tc.tile_pool(name="ps_tr", bufs=2, space="PSUM") as ps_tr_pool,
        ):
            u1_sb = consts.tile([128, 32 * M], f16)     # 8 MB: U1 tiles k
            v2_sb = consts.tile([128, 8 * N], f16)      # 8 MB: V2 tiles j
            ident = consts.tile([128, 128], f32)
            xT = consts.tile([128, 8 * BL], f16)        # xT tiles j

            atb = state.tile([128, 2048], f32)
            q = state.tile([128, 2048], f32)
            t = state.tile([128, 2048], f32)
            cb = state.tile([128, 2048], f32)
            gb = state.tile([128, 2048], f32)
            cl = state.tile([128, 2048], f32)

            pT = small.tile([128, 16 * 128], f16)       # transposed act tiles
            sT = small.tile([128, 8 * BL], f16)         # transposed s tiles
            s32 = small.tile([128, 512], f32)

            # ---- load constants ----
            nc.sync.dma_start(out=ident[:], in_=id_d[:])
            for j in range(8):
                nc.sync.dma_start(out=xT[:, j * BL:(j + 1) * BL], in_=xT16_d[j])
            for j in range(8):
                nc.sync.dma_start(out=v2_sb[:, j * N:(j + 1) * N], in_=v2_d[j])
            for k in range(32):
                nc.sync.dma_start(out=u1_sb[:, k * M:(k + 1) * M], in_=u1_d[k])

            def mm2_bank(lhsT_of_j, out_bank, nb):
                """r[:, bank nb] = sum_m s[b,m] V2[m, bank nb] (folded)."""
                for j in range(8):
                    lhsT = lhsT_of_j(j)
                    base = j * N + 512 * nb
                    nc.tensor.matmul(
                        out_bank[0:64, :], lhsT,
                        v2_sb[:, base:base + 512],
                        start=(j == 0), stop=(j == 7),
                        tile_position=(0, 0), skip_group_check=True)
                    nc.tensor.matmul(
                        out_bank[64:128, :], lhsT,
                        v2_sb[:, base + 2048:base + 2560],
                        start=(j == 0), stop=(j == 7),
                        tile_position=(0, 64), skip_group_check=True)

            def transpose_in(src, dst16):
                """src (128,2048) f32 folded -> dst16 (128, 16*128) f16 tiles."""
                for i in range(16):
                    tr = ps_tr_pool.tile([128, 128], f32, tag="tr")
                    nc.tensor.transpose(
                        tr[:], src[:, 128 * i:128 * (i + 1)], ident[:])
                    nc.scalar.copy(dst16[:, 128 * i:128 * (i + 1)], tr[:])

            def mm1(out_ps):
                """s[b,m] = sum_n t[b,n] U1[n,m] -> folded psum (128,512)."""
                for k in range(32):
                    i = k % 16
                    half = 64 * (k // 16)
                    lhsT = pT[:, 128 * i + half:128 * i + half + 64]
                    nc.tensor.matmul(
                        out_ps[0:64, :], lhsT,
                        u1_sb[:, M * k:M * k + 512],
                        start=(k == 0), stop=(k == 31), tile_position=(0, 0), skip_group_check=True)
                    nc.tensor.matmul(
                        out_ps[64:128, :], lhsT,
                        u1_sb[:, M * k + 512:M * k + 1024],
                        start=(k == 0), stop=(k == 31), tile_position=(0, 64), skip_group_check=True)

            def s_transposes(src32, dst16):
                """src32 (128,512) f32 folded s -> dst16 (128, 8*64) f16."""
                for j in range(8):
                    jj = j % 4
                    half = 64 * (j // 4)
                    tr = ps_tr_pool.tile([128, 64], f32, tag="tr")
                    nc.tensor.transpose(
                        tr[:], src32[half:half + 64, 128 * jj:128 * (jj + 1)],
                        ident[half:half + 64, half:half + 64])
                    nc.scalar.copy(dst16[:, BL * j:BL * (j + 1)], tr[:])

            # ---- prologue: Atb = x @ V2 (stationary = xT tiles) ----
            ps_atb = [ps_r_pool.tile([128, 512], f32, tag="r", name=f"ps_atb{i}") for i in range(4)]
            for j in range(8):
                lhsT = xT[:, BL * j:BL * (j + 1)]
                for nb in range(4):
                    base = j * N + 512 * nb
                    nc.tensor.matmul(ps_atb[nb][0:64, :], lhsT,
                                     v2_sb[:, base:base + 512],
                                     start=(j == 0), stop=(j == 7),
                                     tile_position=(0, 0), skip_group_check=True)
                    nc.tensor.matmul(ps_atb[nb][64:128, :], lhsT,
                                     v2_sb[:, base + 2048:base + 2560],
                                     start=(j == 0), stop=(j == 7),
                                     tile_position=(0, 64), skip_group_check=True)
            for nb in range(4):
                nc.vector.tensor_copy(atb[:, 512 * nb:512 * (nb + 1)],
                                      ps_atb[nb][:])

            # ---- main loop: mm-passes 1..iters ----
            # Pass 1 consumes pT(Atb) with c = Atb; pass idx computes
            # q_idx = c - r_idx, and (if not last) preps pass idx+1 inside
            # the mm2 phase, bank by bank, so the PE never idles long
            # enough for the HAM clock gate to re-throttle.
            # Elementwise in clamp form:
            #   clamp = min(max(q,-l),l); g = Atb + q
            #   t = g - 2*clamp (= Atb + 2*soft(q) - q); c = g - clamp
            transpose_in(atb, pT)
            for idx in range(1, iters + 1):
                ps_s = ps_s_pool.tile([128, 512], f32, tag="s")
                mm1(ps_s)
                nc.scalar.copy(s32[:], ps_s[:])
                s_transposes(s32, sT)
                c_src = atb if idx == 1 else cb
                ps_r = [ps_r_pool.tile([128, 512], f32, tag="r",
                                       name=f"ps_r_{idx}_{i}")
                        for i in range(4)]
                for nb in range(4):
                    sl = slice(512 * nb, 512 * (nb + 1))
                    mm2_bank(lambda j: sT[:, BL * j:BL * (j + 1)],
                             ps_r[nb], nb)
                    nc.vector.tensor_tensor(q[:, sl], c_src[:, sl],
                                            ps_r[nb][:], op=OP.subtract)
                    if idx < iters:
                        nc.vector.tensor_scalar(cl[:, sl], q[:, sl],
                                                -LAMBD, LAMBD,
                                                op0=OP.max, op1=OP.min)
                        nc.vector.tensor_tensor(gb[:, sl], atb[:, sl],
                                                q[:, sl], op=OP.add)
                        nc.vector.scalar_tensor_tensor(
                            t[:, sl], cl[:, sl], -2.0, gb[:, sl],
                            op0=OP.mult, op1=OP.add)
                        for i in range(4 * nb, 4 * nb + 4):
                            tr = ps_tr_pool.tile([128, 128], f32, tag="tr")
                            nc.tensor.transpose(
                                tr[:], t[:, 128 * i:128 * (i + 1)], ident[:])
                            nc.scalar.copy(pT[:, 128 * i:128 * (i + 1)],
                                           tr[:])
                        nc.vector.tensor_tensor(cb[:, sl], gb[:, sl],
                                                cl[:, sl], op=OP.subtract)

            # ---- epilogue: z = soft(q) = q - clamp(q), decoded = z @ wu ----
            nc.vector.tensor_scalar(cl[:], q[:], -LAMBD, LAMBD,
                                    op0=OP.max, op1=OP.min)
            nc.vector.tensor_tensor(t[:], q[:], cl[:], op=OP.subtract)
            nc.sync.dma_start(out=z_d[:, 0:2048], in_=t[0:64, :])
            nc.sync.dma_start(out=z_d[:, 2048:4096], in_=t[64:128, :])

            # decoded: stationary zT tiles, moving wu (streamed from HBM)
            transpose_in(t, pT)
            ps_dec = ps_s_pool.tile([128, 512], f32, tag="s")
            for k in range(32):
                i = k % 16
                half = 64 * (k // 16)
                lhsT = pT[:, 128 * i + half:128 * i + half + 64]
                stage = wustage.tile([128, M], f16, tag="wu")
                nc.sync.dma_start(out=stage[:], in_=wu_d[k])
                nc.tensor.matmul(ps_dec[0:64, :], lhsT, stage[:, 0:512],
                                 start=(k == 0), stop=(k == 31),
                                 tile_position=(0, 0), skip_group_check=True)
                nc.tensor.matmul(ps_dec[64:128, :], lhsT, stage[:, 512:1024],
                                 start=(k == 0), stop=(k == 31),
                                 tile_position=(0, 64), skip_group_check=True)
            dec32 = small.tile([128, 512], f32)
            nc.vector.tensor_copy(dec32[:], ps_dec[:])
            nc.sync.dma_start(out=dec_d[:, 0:512], in_=dec32[0:64, :])
            nc.sync.dma_start(out=dec_d[:, 512:1024], in_=dec32[64:128, :])

    nc.compile()
    return nc


def _host_prep(x, weight):
    """Host-side weight preparation (float64 for the small inversion)."""
    x_flat = np.asarray(x, dtype=np.float32).reshape(B, -1)
    w = np.asarray(weight, dtype=np.float64)
    wu = w / np.linalg.norm(w, axis=1, keepdims=True)        # (4096, 1024)
    G = np.eye(M) + wu.T @ wu
    Gi = np.linalg.inv(G)
    U1 = (wu @ Gi).astype(np.float32)                        # (4096, 1024)
    V2 = np.ascontiguousarray(wu.T).astype(np.float32)       # (1024, 4096)

    u1_16 = U1.astype(np.float16).reshape(32, 128, M)
    v2_16 = V2.astype(np.float16).reshape(8, 128, N)
    wu_16 = wu.astype(np.float16).reshape(32, 128, M)
    ident = np.eye(128, dtype=np.float32)
    return x_flat, u1_16, v2_16, wu_16, ident


def kernel(x, weight):
    from concourse.bass_utils import run_bass_kernel_spmd

    x_flat, u1_16, v2_16, wu_16, ident = _host_prep(x, weight)

    if "nc" not in _PROGRAM_CACHE:
        _PROGRAM_CACHE["nc"] = _build_program(ITERS)
    nc = _PROGRAM_CACHE["nc"]

    in_maps = []
    for c in range(NCORES):
        rows = x_flat[c * BL:(c + 1) * BL]                   # (64, 1024)
        xT16 = np.ascontiguousarray(rows.T).astype(np.float16).reshape(
            8, 128, BL)
        in_maps.append({
            "xT16": xT16,
            "u1": u1_16,
            "v2": v2_16,
            "wu16": wu_16,
            "ident": ident,
        })

    res = run_bass_kernel_spmd(nc, in_maps, list(range(NCORES)))
    z = np.concatenate([res.results[c]["z_out"] for c in range(NCORES)], axis=0)
    dec = np.concatenate([res.results[c]["dec_out"] for c in range(NCORES)],
                         axis=0)
    input_shape = np.asarray(x).shape
    return z.astype(np.float32), dec.astype(np.float32).reshape(input_shape)


# revision 15
# speedup vs baseline: 2.7775x; 1.1911x over previous
"""Trainium2 Bass kernel for nn_MatchingPursuit (ADMM LASSO, 50 iters).

Math (per batch row b of x):
  wu = weight / ||weight||_row                (4096, 1024)
  A = wu.T ; M_inv = inv(A^T A + I_4096)
  Woodbury: p @ M_inv = p - (p @ U1) @ V2,  U1 = wu @ inv(I + wu.T wu),
                                            V2 = wu.T
  ADMM recurrence rewritten in q-form:
    q_1 = Atb - r_1,            r_t = (p_t @ U1) @ V2
    p_1 = Atb
    for t = 2..50:
      v_{t-1} = soft(q_{t-1}, 0.2)
      p_t = Atb + 2 v_{t-1} - q_{t-1}
      q_t = (Atb + v_{t-1}) - r_t
    z = soft(q_50);  decoded = z @ wu

Distribution: pure data-parallel over batch. 512 rows -> 8 cores x 64.
Weights (U1, V2 fp16) replicated and resident in SBUF; fp32 state.

On-chip layout ("folded" batch-major): state tensors are (128, 2048) where
partition = b + 64*(n // 2048), free = n % 2048 for feature n in [0,4096).
Matmuls put weights on the moving operand (N=512) and small transposed
activation tiles (128, 64) fp16 on the stationary side; the two column
halves of the PE array run two concurrent chains (tile_position (0,0) /
(0,64)) writing psum partitions 0:64 / 64:128, which reproduces the folded
layout for free.
"""

import numpy as np

LAMBD = 0.2
ITERS = 50
B = 512
M = 1024  # in_features
N = 4096  # out_features
NCORES = 8
BL = B // NCORES  # 64 rows per core

_PROGRAM_CACHE = {}


def _build_program(iters=ITERS):
    import concourse.bass as bass
    import concourse.bacc as bacc
    import concourse.tile as tile
    import concourse.mybir as mybir

    f32 = mybir.dt.float32
    f16 = mybir.dt.float16
    AF = mybir.ActivationFunctionType
    OP = mybir.AluOpType

    nc = bacc.Bacc(None)

    # ---- DRAM parameters ----
    xT16_d = nc.declare_dram_parameter("xT16", [8, 128, BL], f16, isOutput=False)
    u1_d = nc.declare_dram_parameter("u1", [32, 128, M], f16, isOutput=False)
    v2_d = nc.declare_dram_parameter("v2", [8, 128, N], f16, isOutput=False)
    wu_d = nc.declare_dram_parameter("wu16", [32, 128, M], f16, isOutput=False)
    id_d = nc.declare_dram_parameter("ident", [128, 128], f32, isOutput=False)
    z_d = nc.declare_dram_parameter("z_out", [BL, N], f32, isOutput=True)
    dec_d = nc.declare_dram_parameter("dec_out", [BL, M], f32, isOutput=True)

    with tile.TileContext(nc) as tc:
        with (
            tc.tile_pool(name="consts", bufs=1) as consts,
            tc.tile_pool(name="state", bufs=1) as state,
            tc.tile_pool(name="small", bufs=1) as small,
            tc.tile_pool(name="wustage", bufs=2) as wustage,
            tc.tile_pool(name="ps_s", bufs=2, space="PSUM") as ps_s_pool,
            tc.tile_pool(name="ps_r", bufs=4, space="PSUM") as ps_r_pool,
            tc.tile_pool(name="ps_tr", bufs=2, space="PSUM") as ps_tr_pool,
        ):
            u1_sb = consts.tile([128, 32 * M], f16)     # 8 MB: U1 tiles k
            v2_sb = consts.tile([128, 8 * N], f16)      # 8 MB: V2 tiles j
            ident = consts.tile([128, 128], f32)
            xT = consts.tile([128, 8 * BL], f16)        # xT tiles j

            atb = state.tile([128, 2048], f32)
            q = state.tile([128, 2048], f32)
            t = state.tile([128, 2048], f32)
            cb = state.tile([128, 2048], f32)
            gb = state.tile([128, 2048], f32)
            cl = state.tile([128, 2048], f32)

            pT = small.tile([128, 16 * 128], f16)       # transposed act tiles
            sT = small.tile([128, 8 * BL], f16)         # transposed s tiles
            s16 = small.tile([128, 512], f16)           # fp16 copy of s
            t16 = small.tile([128, 2048], f16)          # fp16 p_t (mm1 input)
            ident16 = small.tile([128, 128], f16)

            # ---- load constants ----
            nc.sync.dma_start(out=ident[:], in_=id_d[:])
            nc.scalar.copy(ident16[:], ident[:])
            for j in range(8):
                nc.sync.dma_start(out=xT[:, j * BL:(j + 1) * BL], in_=xT16_d[j])
            for j in range(8):
                nc.sync.dma_start(out=v2_sb[:, j * N:(j + 1) * N], in_=v2_d[j])
            for k in range(32):
                nc.sync.dma_start(out=u1_sb[:, k * M:(k + 1) * M], in_=u1_d[k])

            def mm2_bank(lhsT_of_j, out_bank, nb):
                """r[:, bank nb] = sum_m s[b,m] V2[m, bank nb] (folded)."""
                for idx_j, j in enumerate(MM2_J_ORDER):
                    lhsT = lhsT_of_j(j)
                    base = j * N + 512 * nb
                    nc.tensor.matmul(
                        out_bank[0:64, :], lhsT,
                        v2_sb[:, base:base + 512],
                        start=(idx_j == 0), stop=(idx_j == 7),
                        tile_position=(0, 0), skip_group_check=True)
                    nc.tensor.matmul(
                        out_bank[64:128, :], lhsT,
                        v2_sb[:, base + 2048:base + 2560],
                        start=(idx_j == 0), stop=(idx_j == 7),
                        tile_position=(0, 64), skip_group_check=True)

            def transpose_in(src, dst16):
                """src (128,2048) f32 folded -> dst16 (128, 16*128) f16 tiles."""
                for i in range(16):
                    tr = ps_tr_pool.tile([128, 128], f32, tag="tr")
                    nc.tensor.transpose(
                        tr[:], src[:, 128 * i:128 * (i + 1)], ident[:])
                    nc.scalar.copy(dst16[:, 128 * i:128 * (i + 1)], tr[:])

            def mm1(out_ps):
                """s[b,m] = sum_n t[b,n] U1[n,m] -> folded psum (128,512)."""
                for k in range(32):
                    i = k % 16
                    half = 64 * (k // 16)
                    lhsT = pT[:, 128 * i + half:128 * i + half + 64]
                    nc.tensor.matmul(
                        out_ps[0:64, :], lhsT,
                        u1_sb[:, M * k:M * k + 512],
                        start=(k == 0), stop=(k == 31), tile_position=(0, 0), skip_group_check=True)
                    nc.tensor.matmul(
                        out_ps[64:128, :], lhsT,
                        u1_sb[:, M * k + 512:M * k + 1024],
                        start=(k == 0), stop=(k == 31), tile_position=(0, 64), skip_group_check=True)

            MM2_J_ORDER = [0, 4, 1, 5, 2, 6, 3, 7]

            def s_stage(ps_s_tile):
                """psum s -> s16 (sliced fp16 copies) -> sT tiles, in an
                order that lets mm2 start after the first slice."""
                for jj in range(4):
                    nc.scalar.copy(s16[:, 128 * jj:128 * (jj + 1)],
                                   ps_s_tile[:, 128 * jj:128 * (jj + 1)])
                    for j in (jj, jj + 4):
                        half = 64 * (j // 4)
                        tr = ps_tr_pool.tile([128, 64], f16, tag="tr",
                                             name=f"trs{j}")
                        nc.tensor.transpose(
                            tr[:],
                            s16[half:half + 64, 128 * jj:128 * (jj + 1)],
                            ident16[half:half + 64, half:half + 64])
                        nc.scalar.copy(sT[:, BL * j:BL * (j + 1)], tr[:])

            # ---- prologue: Atb = x @ V2 (stationary = xT tiles) ----
            ps_atb = [ps_r_pool.tile([128, 512], f32, tag="r", name=f"ps_atb{i}") for i in range(4)]
            for j in range(8):
                lhsT = xT[:, BL * j:BL * (j + 1)]
                for nb in range(4):
                    base = j * N + 512 * nb
                    nc.tensor.matmul(ps_atb[nb][0:64, :], lhsT,
                                     v2_sb[:, base:base + 512],
                                     start=(j == 0), stop=(j == 7),
                                     tile_position=(0, 0), skip_group_check=True)
                    nc.tensor.matmul(ps_atb[nb][64:128, :], lhsT,
                                     v2_sb[:, base + 2048:base + 2560],
                                     start=(j == 0), stop=(j == 7),
                                     tile_position=(0, 64), skip_group_check=True)
            for nb in range(4):
                nc.vector.tensor_copy(atb[:, 512 * nb:512 * (nb + 1)],
                                      ps_atb[nb][:])

            # ---- main loop: mm-passes 1..iters ----
            # Pass 1 consumes pT(Atb) with c = Atb; pass idx computes
            # q_idx = c - r_idx, and (if not last) preps pass idx+1 inside
            # the mm2 phase, bank by bank, so the PE never idles long
            # enough for the HAM clock gate to re-throttle.
            # Elementwise in clamp form:
            #   clamp = min(max(q,-l),l); g = Atb + q
            #   t = g - 2*clamp (= Atb + 2*soft(q) - q); c = g - clamp
            transpose_in(atb, pT)
            for idx in range(1, iters + 1):
                ps_s = ps_s_pool.tile([128, 512], f32, tag="s")
                mm1(ps_s)
                s_stage(ps_s)
                c_src = atb if idx == 1 else cb
                ps_r = [ps_r_pool.tile([128, 512], f32, tag="r",
                                       name=f"ps_r_{idx}_{i}")
                        for i in range(4)]
                for nb in range(4):
                    sl = slice(512 * nb, 512 * (nb + 1))
                    mm2_bank(lambda j: sT[:, BL * j:BL * (j + 1)],
                             ps_r[nb], nb)
                    nc.vector.tensor_tensor(q[:, sl], c_src[:, sl],
                                            ps_r[nb][:], op=OP.subtract)
                    if idx < iters:
                        nc.vector.tensor_scalar(cl[:, sl], q[:, sl],
                                                -LAMBD, LAMBD,
                                                op0=OP.max, op1=OP.min)
                        nc.vector.tensor_tensor(gb[:, sl], atb[:, sl],
                                                q[:, sl], op=OP.add)
                        nc.vector.scalar_tensor_tensor(
                            t16[:, sl], cl[:, sl], -2.0, gb[:, sl],
                            op0=OP.mult, op1=OP.add)
                        for i in range(4 * nb, 4 * nb + 4):
                            tr = ps_tr_pool.tile([128, 128], f16, tag="tr")
                            nc.tensor.transpose(
                                tr[:], t16[:, 128 * i:128 * (i + 1)],
                                ident16[:])
                            nc.scalar.copy(pT[:, 128 * i:128 * (i + 1)],
                                           tr[:])
                        nc.vector.tensor_tensor(cb[:, sl], gb[:, sl],
                                                cl[:, sl], op=OP.subtract)

            # ---- epilogue: z = soft(q) = q - clamp(q), decoded = z @ wu ----
            nc.vector.tensor_scalar(cl[:], q[:], -LAMBD, LAMBD,
                                    op0=OP.max, op1=OP.min)
            nc.vector.tensor_tensor(t[:], q[:], cl[:], op=OP.subtract)
            nc.sync.dma_start(out=z_d[:, 0:2048], in_=t[0:64, :])
            nc.sync.dma_start(out=z_d[:, 2048:4096], in_=t[64:128, :])

            # decoded: stationary zT tiles, moving wu (streamed from HBM)
            transpose_in(t, pT)
            ps_dec = ps_s_pool.tile([128, 512], f32, tag="s")
            for k in range(32):
                i = k % 16
                half = 64 * (k // 16)
                lhsT = pT[:, 128 * i + half:128 * i + half + 64]
                stage = wustage.tile([128, M], f16, tag="wu")
                nc.sync.dma_start(out=stage[:], in_=wu_d[k])
                nc.tensor.matmul(ps_dec[0:64, :], lhsT, stage[:, 0:512],
                                 start=(k == 0), stop=(k == 31),
                                 tile_position=(0, 0), skip_group_check=True)
                nc.tensor.matmul(ps_dec[64:128, :], lhsT, stage[:, 512:1024],
                                 start=(k == 0), stop=(k == 31),
                                 tile_position=(0, 64), skip_group_check=True)
            dec32 = small.tile([128, 512], f32)
            nc.vector.tensor_copy(dec32[:], ps_dec[:])
            nc.sync.dma_start(out=dec_d[:, 0:512], in_=dec32[0:64, :])
            nc.sync.dma_start(out=dec_d[:, 512:1024], in_=dec32[64:128, :])

    nc.compile()
    return nc


def _host_prep(x, weight):
    """Host-side weight preparation (float64 for the small inversion)."""
    x_flat = np.asarray(x, dtype=np.float32).reshape(B, -1)
    w = np.asarray(weight, dtype=np.float64)
    wu = w / np.linalg.norm(w, axis=1, keepdims=True)        # (4096, 1024)
    G = np.eye(M) + wu.T @ wu
    Gi = np.linalg.inv(G)
    U1 = (wu @ Gi).astype(np.float32)                        # (4096, 1024)
    V2 = np.ascontiguousarray(wu.T).astype(np.float32)       # (1024, 4096)

    u1_16 = U1.astype(np.float16).reshape(32, 128, M)
    v2_16 = V2.astype(np.float16).reshape(8, 128, N)
    wu_16 = wu.astype(np.float16).reshape(32, 128, M)
    ident = np.eye(128, dtype=np.float32)
    return x_flat, u1_16, v2_16, wu_16, ident


def kernel(x, weight):
    from concourse.bass_utils import run_bass_kernel_spmd

    x_flat, u1_16, v2_16, wu_16, ident = _host_prep(x, weight)

    if "nc" not in _PROGRAM_CACHE:
        _PROGRAM_CACHE["nc"] = _build_program(ITERS)
    nc = _PROGRAM_CACHE["nc"]

    in_maps = []
    for c in range(NCORES):
        rows = x_flat[c * BL:(c + 1) * BL]                   # (64, 1024)
        xT16 = np.ascontiguousarray(rows.T).astype(np.float16).reshape(
            8, 128, BL)
        in_maps.append({
            "xT16": xT16,
            "u1": u1_16,
            "v2": v2_16,
            "wu16": wu_16,
            "ident": ident,
        })

    res = run_bass_kernel_spmd(nc, in_maps, list(range(NCORES)))
    z = np.concatenate([res.results[c]["z_out"] for c in range(NCORES)], axis=0)
    dec = np.concatenate([res.results[c]["dec_out"] for c in range(NCORES)],
                         axis=0)
    input_shape = np.asarray(x).shape
    return z.astype(np.float32), dec.astype(np.float32).reshape(input_shape)


# revision 17
# speedup vs baseline: 2.9298x; 1.0548x over previous
"""Trainium2 Bass kernel for nn_MatchingPursuit (ADMM LASSO, 50 iters).

Math (per batch row b of x):
  wu = weight / ||weight||_row                (4096, 1024)
  A = wu.T ; M_inv = inv(A^T A + I_4096)
  Woodbury: p @ M_inv = p - (p @ U1) @ V2,  U1 = wu @ inv(I + wu.T wu),
                                            V2 = wu.T
  ADMM recurrence rewritten in q-form:
    q_1 = Atb - r_1,            r_t = (p_t @ U1) @ V2
    p_1 = Atb
    for t = 2..50:
      v_{t-1} = soft(q_{t-1}, 0.2)
      p_t = Atb + 2 v_{t-1} - q_{t-1}
      q_t = (Atb + v_{t-1}) - r_t
    z = soft(q_50);  decoded = z @ wu

Distribution: pure data-parallel over batch. 512 rows -> 8 cores x 64.
Weights (U1, V2 fp16) replicated and resident in SBUF; fp32 state.

On-chip layout ("folded" batch-major): state tensors are (128, 2048) where
partition = b + 64*(n // 2048), free = n % 2048 for feature n in [0,4096).
Matmuls put weights on the moving operand (N=512) and small transposed
activation tiles (128, 64) fp16 on the stationary side; the two column
halves of the PE array run two concurrent chains (tile_position (0,0) /
(0,64)) writing psum partitions 0:64 / 64:128, which reproduces the folded
layout for free.
"""

import numpy as np

LAMBD = 0.2
ITERS = 50
B = 512
M = 1024  # in_features
N = 4096  # out_features
NCORES = 8
BL = B // NCORES  # 64 rows per core

_PROGRAM_CACHE = {}


def _build_program(iters=ITERS):
    import concourse.bass as bass
    import concourse.bacc as bacc
    import concourse.tile as tile
    import concourse.mybir as mybir

    f32 = mybir.dt.float32
    f16 = mybir.dt.float16
    AF = mybir.ActivationFunctionType
    OP = mybir.AluOpType

    nc = bacc.Bacc(None)

    # ---- DRAM parameters ----
    xT16_d = nc.declare_dram_parameter("xT16", [8, 128, BL], f16, isOutput=False)
    u1_d = nc.declare_dram_parameter("u1", [32, 128, M], f16, isOutput=False)
    v2_d = nc.declare_dram_parameter("v2", [8, 128, N], f16, isOutput=False)
    wu_d = nc.declare_dram_parameter("wu16", [32, 128, M], f16, isOutput=False)
    id_d = nc.declare_dram_parameter("ident", [128, 128], f32, isOutput=False)
    z_d = nc.declare_dram_parameter("z_out", [BL, N], f32, isOutput=True)
    dec_d = nc.declare_dram_parameter("dec_out", [BL, M], f32, isOutput=True)

    with tile.TileContext(nc) as tc:
        with (
            tc.tile_pool(name="consts", bufs=1) as consts,
            tc.tile_pool(name="state", bufs=1) as state,
            tc.tile_pool(name="small", bufs=1) as small,
            tc.tile_pool(name="wustage", bufs=2) as wustage,
            tc.tile_pool(name="ps_s", bufs=1, space="PSUM") as ps_s_pool,
            tc.tile_pool(name="ps_r", bufs=4, space="PSUM") as ps_r_pool,
            tc.tile_pool(name="ps_tr", bufs=3, space="PSUM") as ps_tr_pool,
        ):
            u1_sb = consts.tile([128, 32 * M], f16)     # 8 MB: U1 tiles k
            v2_sb = consts.tile([128, 8 * N], f16)      # 8 MB: V2 tiles j
            ident = consts.tile([128, 128], f32)
            xT = consts.tile([128, 8 * BL], f16)        # xT tiles j

            atb = state.tile([128, 2048], f32)
            q = state.tile([128, 2048], f32)
            t = state.tile([128, 2048], f32)
            cb = state.tile([128, 2048], f32)
            gb = state.tile([128, 2048], f32)
            cl = state.tile([128, 2048], f32)

            pT = small.tile([128, 16 * 128], f16)       # transposed act tiles
            sT = small.tile([128, 8 * BL], f16)         # transposed s tiles
            s16 = small.tile([128, 512], f16)           # fp16 copy of s
            t16 = small.tile([128, 2048], f16)          # fp16 p_t (mm1 input)
            ident16 = small.tile([128, 128], f16)

            # ---- load constants ----
            nc.sync.dma_start(out=ident[:], in_=id_d[:])
            nc.scalar.copy(ident16[:], ident[:])
            for j in range(8):
                nc.sync.dma_start(out=xT[:, j * BL:(j + 1) * BL], in_=xT16_d[j])
            for j in range(8):
                nc.sync.dma_start(out=v2_sb[:, j * N:(j + 1) * N], in_=v2_d[j])
            for k in range(32):
                nc.sync.dma_start(out=u1_sb[:, k * M:(k + 1) * M], in_=u1_d[k])

            def mm2_bank(lhsT_of_j, out_bank, nb):
                """r[:, bank nb] = sum_m s[b,m] V2[m, bank nb] (folded)."""
                for idx_j, j in enumerate(MM2_J_ORDER):
                    lhsT = lhsT_of_j(j)
                    base = j * N + 512 * nb
                    nc.tensor.matmul(
                        out_bank[0:64, :], lhsT,
                        v2_sb[:, base:base + 512],
                        start=(idx_j == 0), stop=(idx_j == 7),
                        tile_position=(0, 0), skip_group_check=True)
                    nc.tensor.matmul(
                        out_bank[64:128, :], lhsT,
                        v2_sb[:, base + 2048:base + 2560],
                        start=(idx_j == 0), stop=(idx_j == 7),
                        tile_position=(0, 64), skip_group_check=True)

            def transpose_in(src, dst16):
                """src (128,2048) f32 folded -> dst16 (128, 16*128) f16 tiles."""
                for i in range(16):
                    tr = ps_tr_pool.tile([128, 128], f32, tag="tr")
                    nc.tensor.transpose(
                        tr[:], src[:, 128 * i:128 * (i + 1)], ident[:])
                    nc.scalar.copy(dst16[:, 128 * i:128 * (i + 1)], tr[:])

            def mm1(out_ps):
                """s[b,m] = sum_n t[b,n] U1[n,m] -> folded psum (128,512)."""
                for k in range(32):
                    i = k % 16
                    half = 64 * (k // 16)
                    lhsT = pT[:, 128 * i + half:128 * i + half + 64]
                    nc.tensor.matmul(
                        out_ps[0:64, :], lhsT,
                        u1_sb[:, M * k:M * k + 512],
                        start=(k == 0), stop=(k == 31), tile_position=(0, 0), skip_group_check=True)
                    nc.tensor.matmul(
                        out_ps[64:128, :], lhsT,
                        u1_sb[:, M * k + 512:M * k + 1024],
                        start=(k == 0), stop=(k == 31), tile_position=(0, 64), skip_group_check=True)

            MM2_J_ORDER = [0, 4, 1, 5, 2, 6, 3, 7]

            def s_stage(ps_s_tile):
                """psum s -> s16 (sliced fp16 copies) -> sT tiles, in an
                order that lets mm2 start after the first slice."""
                for jj in range(4):
                    nc.scalar.copy(s16[:, 128 * jj:128 * (jj + 1)],
                                   ps_s_tile[:, 128 * jj:128 * (jj + 1)])
                    for j in (jj, jj + 4):
                        half = 64 * (j // 4)
                        tr = ps_tr_pool.tile([128, 64], f16, tag="tr",
                                             name=f"trs{j}")
                        nc.tensor.transpose(
                            tr[:],
                            s16[half:half + 64, 128 * jj:128 * (jj + 1)],
                            ident16[half:half + 64, half:half + 64])
                        nc.scalar.copy(sT[:, BL * j:BL * (j + 1)], tr[:])

            # ---- prologue: Atb = x @ V2 (stationary = xT tiles) ----
            ps_atb = [ps_r_pool.tile([128, 512], f32, tag="r", name=f"ps_atb{i}") for i in range(4)]
            for j in range(8):
                lhsT = xT[:, BL * j:BL * (j + 1)]
                for nb in range(4):
                    base = j * N + 512 * nb
                    nc.tensor.matmul(ps_atb[nb][0:64, :], lhsT,
                                     v2_sb[:, base:base + 512],
                                     start=(j == 0), stop=(j == 7),
                                     tile_position=(0, 0), skip_group_check=True)
                    nc.tensor.matmul(ps_atb[nb][64:128, :], lhsT,
                                     v2_sb[:, base + 2048:base + 2560],
                                     start=(j == 0), stop=(j == 7),
                                     tile_position=(0, 64), skip_group_check=True)
            for nb in range(4):
                nc.vector.tensor_copy(atb[:, 512 * nb:512 * (nb + 1)],
                                      ps_atb[nb][:])

            # ---- main loop: mm-passes 1..iters ----
            # Pass 1 consumes pT(Atb) with c = Atb; pass idx computes
            # q_idx = c - r_idx, and (if not last) preps pass idx+1 inside
            # the mm2 phase, bank by bank, so the PE never idles long
            # enough for the HAM clock gate to re-throttle.
            # Elementwise in clamp form:
            #   clamp = min(max(q,-l),l); g = Atb + q
            #   t = g - 2*clamp (= Atb + 2*soft(q) - q); c = g - clamp
            # The transpose groups (PE) are staggered so each lands in the
            # PE stream only after its DVE chain has had time to finish:
            # banks 0/1 late in this mm2 phase, banks 2/3 inside the NEXT
            # pass's mm1 (the mm1 k-order is arranged to need them last).
            MM1_ORDER = (list(range(0, 8)) + list(range(16, 24)) +
                         list(range(8, 12)) + list(range(24, 28)) +
                         list(range(12, 16)) + list(range(28, 32)))

            def mm1_part(out_ps, lo, hi):
                for pos in range(lo, hi):
                    k = MM1_ORDER[pos]
                    i = k % 16
                    half = 64 * (k // 16)
                    lhsT = pT[:, 128 * i + half:128 * i + half + 64]
                    nc.tensor.matmul(
                        out_ps[0:64, :], lhsT,
                        u1_sb[:, M * k:M * k + 512],
                        start=(pos == 0), stop=(pos == 31),
                        tile_position=(0, 0), skip_group_check=True)
                    nc.tensor.matmul(
                        out_ps[64:128, :], lhsT,
                        u1_sb[:, M * k + 512:M * k + 1024],
                        start=(pos == 0), stop=(pos == 31),
                        tile_position=(0, 64), skip_group_check=True)

            def tr_group(nb):
                for i in range(4 * nb, 4 * nb + 4):
                    tr = ps_tr_pool.tile([128, 128], f16, tag="tr",
                                         name=f"trp{i}")
                    nc.tensor.transpose(
                        tr[:], t16[:, 128 * i:128 * (i + 1)], ident16[:])
                    nc.scalar.copy(pT[:, 128 * i:128 * (i + 1)], tr[:])

            def dve_chain(nb, c_src, prep):
                sl = slice(512 * nb, 512 * (nb + 1))
                nc.vector.tensor_tensor(q[:, sl], c_src[:, sl],
                                        ps_r[nb][:], op=OP.subtract)
                if prep:
                    nc.vector.tensor_scalar(cl[:, sl], q[:, sl],
                                            -LAMBD, LAMBD,
                                            op0=OP.max, op1=OP.min)
                    nc.vector.tensor_tensor(gb[:, sl], atb[:, sl],
                                            q[:, sl], op=OP.add)
                    nc.vector.scalar_tensor_tensor(
                        t16[:, sl], cl[:, sl], -2.0, gb[:, sl],
                        op0=OP.mult, op1=OP.add)

            transpose_in(atb, pT)
            have_pending = False
            for idx in range(1, iters + 1):
                prep = idx < iters
                ps_s = ps_s_pool.tile([128, 512], f32, tag="s")
                mm1_part(ps_s, 0, 16)
                if have_pending:
                    tr_group(2)
                mm1_part(ps_s, 16, 24)
                if have_pending:
                    tr_group(3)
                mm1_part(ps_s, 24, 32)
                s_stage(ps_s)
                c_src = atb if idx == 1 else cb
                ps_r = [ps_r_pool.tile([128, 512], f32, tag="r",
                                       name=f"ps_r_{idx}_{i}")
                        for i in range(4)]
                lhsT_of_j = lambda j: sT[:, BL * j:BL * (j + 1)]
                mm2_bank(lhsT_of_j, ps_r[0], 0)
                dve_chain(0, c_src, prep)
                mm2_bank(lhsT_of_j, ps_r[1], 1)
                dve_chain(1, c_src, prep)
                mm2_bank(lhsT_of_j, ps_r[2], 2)
                dve_chain(2, c_src, prep)
                if prep:
                    tr_group(0)
                mm2_bank(lhsT_of_j, ps_r[3], 3)
                dve_chain(3, c_src, prep)
                if prep:
                    tr_group(1)
                    for nb in range(4):
                        sl = slice(512 * nb, 512 * (nb + 1))
                        nc.vector.tensor_tensor(cb[:, sl], gb[:, sl],
                                                cl[:, sl], op=OP.subtract)
                have_pending = prep

            # ---- epilogue: z = soft(q) = q - clamp(q), decoded = z @ wu ----
            nc.vector.tensor_scalar(cl[:], q[:], -LAMBD, LAMBD,
                                    op0=OP.max, op1=OP.min)
            nc.vector.tensor_tensor(t[:], q[:], cl[:], op=OP.subtract)
            nc.sync.dma_start(out=z_d[:, 0:2048], in_=t[0:64, :])
            nc.sync.dma_start(out=z_d[:, 2048:4096], in_=t[64:128, :])

            # decoded: stationary zT tiles, moving wu (streamed from HBM)
            transpose_in(t, pT)
            ps_dec = ps_s_pool.tile([128, 512], f32, tag="s")
            for k in range(32):
                i = k % 16
                half = 64 * (k // 16)
                lhsT = pT[:, 128 * i + half:128 * i + half + 64]
                stage = wustage.tile([128, M], f16, tag="wu")
                nc.sync.dma_start(out=stage[:], in_=wu_d[k])
                nc.tensor.matmul(ps_dec[0:64, :], lhsT, stage[:, 0:512],
                                 start=(k == 0), stop=(k == 31),
                                 tile_position=(0, 0), skip_group_check=True)
                nc.tensor.matmul(ps_dec[64:128, :], lhsT, stage[:, 512:1024],
                                 start=(k == 0), stop=(k == 31),
                                 tile_position=(0, 64), skip_group_check=True)
            dec32 = small.tile([128, 512], f32)
            nc.vector.tensor_copy(dec32[:], ps_dec[:])
            nc.sync.dma_start(out=dec_d[:, 0:512], in_=dec32[0:64, :])
            nc.sync.dma_start(out=dec_d[:, 512:1024], in_=dec32[64:128, :])

    nc.compile()
    return nc


def _host_prep(x, weight):
    """Host-side weight preparation (float64 for the small inversion)."""
    x_flat = np.asarray(x, dtype=np.float32).reshape(B, -1)
    w = np.asarray(weight, dtype=np.float64)
    wu = w / np.linalg.norm(w, axis=1, keepdims=True)        # (4096, 1024)
    G = np.eye(M) + wu.T @ wu
    Gi = np.linalg.inv(G)
    U1 = (wu @ Gi).astype(np.float32)                        # (4096, 1024)
    V2 = np.ascontiguousarray(wu.T).astype(np.float32)       # (1024, 4096)

    u1_16 = U1.astype(np.float16).reshape(32, 128, M)
    v2_16 = V2.astype(np.float16).reshape(8, 128, N)
    wu_16 = wu.astype(np.float16).reshape(32, 128, M)
    ident = np.eye(128, dtype=np.float32)
    return x_flat, u1_16, v2_16, wu_16, ident


def kernel(x, weight):
    from concourse.bass_utils import run_bass_kernel_spmd

    x_flat, u1_16, v2_16, wu_16, ident = _host_prep(x, weight)

    if "nc" not in _PROGRAM_CACHE:
        _PROGRAM_CACHE["nc"] = _build_program(ITERS)
    nc = _PROGRAM_CACHE["nc"]

    in_maps = []
    for c in range(NCORES):
        rows = x_flat[c * BL:(c + 1) * BL]                   # (64, 1024)
        xT16 = np.ascontiguousarray(rows.T).astype(np.float16).reshape(
            8, 128, BL)
        in_maps.append({
            "xT16": xT16,
            "u1": u1_16,
            "v2": v2_16,
            "wu16": wu_16,
            "ident": ident,
        })

    res = run_bass_kernel_spmd(nc, in_maps, list(range(NCORES)))
    z = np.concatenate([res.results[c]["z_out"] for c in range(NCORES)], axis=0)
    dec = np.concatenate([res.results[c]["dec_out"] for c in range(NCORES)],
                         axis=0)
    input_shape = np.asarray(x).shape
    return z.astype(np.float32), dec.astype(np.float32).reshape(input_shape)


# revision 21
# speedup vs baseline: 3.6380x; 1.2417x over previous
"""Trainium2 Bass kernel for nn_MatchingPursuit (ADMM LASSO, 50 iters).

Math (per batch row b of x):
  wu = weight / ||weight||_row                (4096, 1024)
  A = wu.T ; M_inv = inv(A^T A + I_4096)
  Woodbury: p @ M_inv = p - (p @ U1) @ V2,  U1 = wu @ inv(I + wu.T wu),
                                            V2 = wu.T
  ADMM recurrence rewritten in q-form:
    q_1 = Atb - r_1,            r_t = (p_t @ U1) @ V2
    p_1 = Atb
    for t = 2..50:
      v_{t-1} = soft(q_{t-1}, 0.2)
      p_t = Atb + 2 v_{t-1} - q_{t-1}
      q_t = (Atb + v_{t-1}) - r_t
    z = soft(q_50);  decoded = z @ wu

Distribution: pure data-parallel over batch. 512 rows -> 8 cores x 64.
Weights (U1, V2 fp16) replicated and resident in SBUF; fp32 state.

On-chip layout ("folded" batch-major): state tensors are (128, 2048) where
partition = b + 64*(n // 2048), free = n % 2048 for feature n in [0,4096).
Matmuls put weights on the moving operand (N=512) and small transposed
activation tiles (128, 64) fp16 on the stationary side; the two column
halves of the PE array run two concurrent chains (tile_position (0,0) /
(0,64)) writing psum partitions 0:64 / 64:128, which reproduces the folded
layout for free.
"""

import numpy as np

LAMBD = 0.2
ITERS = 50
B = 512
M = 1024  # in_features
N = 4096  # out_features
NCORES = 8
BL = B // NCORES  # 64 rows per core

_PROGRAM_CACHE = {}


def _build_program(iters=ITERS):
    import concourse.bass as bass
    import concourse.bacc as bacc
    import concourse.tile as tile
    import concourse.mybir as mybir

    f32 = mybir.dt.float32
    f16 = mybir.dt.float16
    AF = mybir.ActivationFunctionType
    OP = mybir.AluOpType

    nc = bacc.Bacc(None)

    # ---- DRAM parameters ----
    xT16_d = nc.declare_dram_parameter("xT16", [8, 128, BL], f16, isOutput=False)
    u1_d = nc.declare_dram_parameter("u1", [32, 128, M], f16, isOutput=False)
    v2_d = nc.declare_dram_parameter("v2", [8, 128, N], f16, isOutput=False)
    g_d = nc.declare_dram_parameter("g16", [8, 128, M], f16, isOutput=False)
    id_d = nc.declare_dram_parameter("ident", [128, 128], f32, isOutput=False)
    z_d = nc.declare_dram_parameter("z_out", [BL, N], f32, isOutput=True)
    dec_d = nc.declare_dram_parameter("dec_out", [BL, M], f32, isOutput=True)

    with tile.TileContext(nc) as tc:
        with (
            tc.tile_pool(name="consts", bufs=1) as consts,
            tc.tile_pool(name="state", bufs=1) as state,
            tc.tile_pool(name="small", bufs=1) as small,
            tc.tile_pool(name="wustage", bufs=4) as wustage,
            tc.tile_pool(name="ps_s", bufs=1, space="PSUM") as ps_s_pool,
            tc.tile_pool(name="ps_r", bufs=4, space="PSUM") as ps_r_pool,
            tc.tile_pool(name="ps_tr", bufs=3, space="PSUM") as ps_tr_pool,
        ):
            u1_sb = consts.tile([128, 32 * M], f16)     # 8 MB: U1 tiles k
            v2_sb = consts.tile([128, 8 * N], f16)      # 8 MB: V2 tiles j
            ident = consts.tile([128, 128], f32)
            xT = consts.tile([128, 8 * BL], f16)        # xT tiles j

            atb = state.tile([128, 2048], f32)
            q = state.tile([128, 2048], f32)
            t = state.tile([128, 2048], f32)
            cb = state.tile([128, 2048], f32)
            gb = state.tile([128, 2048], f32)
            cl = state.tile([128, 2048], f32)

            pT = small.tile([128, 16 * 128], f16)       # transposed act tiles
            sT = small.tile([128, 8 * BL], f16)         # transposed s tiles
            s16 = small.tile([128, 512], f16)           # fp16 copy of s
            t16 = small.tile([128, 2048], f16)          # fp16 p_t (mm1 input)
            ident16 = small.tile([128, 128], f16)

            # ---- load constants ----
            nc.sync.dma_start(out=ident[:], in_=id_d[:])
            nc.scalar.copy(ident16[:], ident[:])
            for j in range(8):
                nc.sync.dma_start(out=xT[:, j * BL:(j + 1) * BL], in_=xT16_d[j])
            for j in range(8):
                nc.sync.dma_start(out=v2_sb[:, j * N:(j + 1) * N], in_=v2_d[j])
            for k in range(32):
                nc.sync.dma_start(out=u1_sb[:, k * M:(k + 1) * M], in_=u1_d[k])

            def mm2_bank(lhsT_of_j, out_bank, nb):
                """r[:, bank nb] = sum_m s[b,m] V2[m, bank nb] (folded)."""
                for idx_j, j in enumerate(MM2_J_ORDER):
                    lhsT = lhsT_of_j(j)
                    base = j * N + 512 * nb
                    nc.tensor.matmul(
                        out_bank[0:64, :], lhsT,
                        v2_sb[:, base:base + 512],
                        start=(idx_j == 0), stop=(idx_j == 7),
                        tile_position=(0, 0), skip_group_check=True)
                    nc.tensor.matmul(
                        out_bank[64:128, :], lhsT,
                        v2_sb[:, base + 2048:base + 2560],
                        start=(idx_j == 0), stop=(idx_j == 7),
                        tile_position=(0, 64), skip_group_check=True)

            def transpose_in(src, dst16):
                """src (128,2048) f32 folded -> dst16 (128, 16*128) f16 tiles."""
                for g in range(4):
                    bank = ps_tr_pool.tile([128, 512], f32, tag="trb",
                                           name=f"tin{g}")
                    for ii in range(4):
                        i = 4 * g + ii
                        nc.tensor.transpose(
                            bank[:, 128 * ii:128 * (ii + 1)],
                            src[:, 128 * i:128 * (i + 1)], ident[:])
                    nc.scalar.copy(dst16[:, 512 * g:512 * (g + 1)], bank[:])

            MM2_J_ORDER = [0, 4, 1, 5, 2, 6, 3, 7]

            def s_stage(ps_s_tile):
                """psum s -> s16 (fp16) -> sT tiles.  A full-128 transpose
                of s16[:, 128jj:128jj+128] yields the m-chunk pair
                (jj, jj+4) side by side; sT layout is jj-major:
                lhsT(j) = sT[:, 128*(j%4) + 64*(j//4) :][:64]."""
                bank = ps_tr_pool.tile([128, 512], f16, tag="trb",
                                       name="strb")
                for jj in range(4):
                    nc.scalar.copy(s16[:, 128 * jj:128 * (jj + 1)],
                                   ps_s_tile[:, 128 * jj:128 * (jj + 1)])
                    nc.tensor.transpose(
                        bank[:, 128 * jj:128 * (jj + 1)],
                        s16[:, 128 * jj:128 * (jj + 1)], ident16[:])
                    if jj == 1:
                        nc.scalar.copy(sT[:, 0:256], bank[:, 0:256])
                nc.scalar.copy(sT[:, 256:512], bank[:, 256:512])

            # ---- prologue: Atb = x @ V2 (stationary = xT tiles) ----
            ps_atb = [ps_r_pool.tile([128, 512], f32, tag="r", name=f"ps_atb{i}") for i in range(4)]
            for j in range(8):
                lhsT = xT[:, BL * j:BL * (j + 1)]
                for nb in range(4):
                    base = j * N + 512 * nb
                    nc.tensor.matmul(ps_atb[nb][0:64, :], lhsT,
                                     v2_sb[:, base:base + 512],
                                     start=(j == 0), stop=(j == 7),
                                     tile_position=(0, 0), skip_group_check=True)
                    nc.tensor.matmul(ps_atb[nb][64:128, :], lhsT,
                                     v2_sb[:, base + 2048:base + 2560],
                                     start=(j == 0), stop=(j == 7),
                                     tile_position=(0, 64), skip_group_check=True)
            for nb in range(4):
                nc.vector.tensor_copy(atb[:, 512 * nb:512 * (nb + 1)],
                                      ps_atb[nb][:])

            # ---- main loop: mm-passes 1..iters ----
            # Pass 1 consumes pT(Atb) with c = Atb; pass idx computes
            # q_idx = c - r_idx, and (if not last) preps pass idx+1 inside
            # the mm2 phase, bank by bank, so the PE never idles long
            # enough for the HAM clock gate to re-throttle.
            # Elementwise in clamp form:
            #   clamp = min(max(q,-l),l); g = Atb + q
            #   t = g - 2*clamp (= Atb + 2*soft(q) - q); c = g - clamp
            # The transpose groups (PE) are staggered so each lands in the
            # PE stream only after its DVE chain has had time to finish:
            # banks 0/1 late in this mm2 phase, banks 2/3 inside the NEXT
            # pass's mm1 (the mm1 k-order is arranged to need them last).
            MM1_ORDER = (list(range(0, 8)) + list(range(16, 24)) +
                         list(range(8, 12)) + list(range(24, 28)) +
                         list(range(12, 16)) + list(range(28, 32)))

            def mm1_part(out_ps, lo, hi):
                for pos in range(lo, hi):
                    k = MM1_ORDER[pos]
                    i = k % 16
                    half = 64 * (k // 16)
                    lhsT = pT[:, 128 * i + half:128 * i + half + 64]
                    nc.tensor.matmul(
                        out_ps[0:64, :], lhsT,
                        u1_sb[:, M * k:M * k + 512],
                        start=(pos == 0), stop=(pos == 31),
                        tile_position=(0, 0), skip_group_check=True)
                    nc.tensor.matmul(
                        out_ps[64:128, :], lhsT,
                        u1_sb[:, M * k + 512:M * k + 1024],
                        start=(pos == 0), stop=(pos == 31),
                        tile_position=(0, 64), skip_group_check=True)

            def tr_group(nb):
                bank = ps_tr_pool.tile([128, 512], f16, tag="trb",
                                       name=f"trb{nb}")
                for ii in range(4):
                    i = 4 * nb + ii
                    nc.tensor.transpose(
                        bank[:, 128 * ii:128 * (ii + 1)],
                        t16[:, 128 * i:128 * (i + 1)], ident16[:])
                nc.scalar.copy(pT[:, 512 * nb:512 * (nb + 1)], bank[:])

            def dve_chain(nb, c_src, prep):
                sl = slice(512 * nb, 512 * (nb + 1))
                nc.vector.tensor_tensor(q[:, sl], c_src[:, sl],
                                        ps_r[nb][:], op=OP.subtract)
                if prep:
                    nc.vector.tensor_scalar(cl[:, sl], q[:, sl],
                                            -LAMBD, LAMBD,
                                            op0=OP.max, op1=OP.min)
                    nc.vector.tensor_tensor(gb[:, sl], atb[:, sl],
                                            q[:, sl], op=OP.add)
                    nc.vector.scalar_tensor_tensor(
                        t16[:, sl], cl[:, sl], -2.0, gb[:, sl],
                        op0=OP.mult, op1=OP.add)

            transpose_in(atb, pT)
            have_pending = False
            for idx in range(1, iters + 1):
                prep = idx < iters
                ps_s = ps_s_pool.tile([128, 512], f32, tag="s")
                mm1_part(ps_s, 0, 16)
                if have_pending:
                    tr_group(2)
                mm1_part(ps_s, 16, 24)
                if have_pending:
                    tr_group(3)
                mm1_part(ps_s, 24, 32)
                s_stage(ps_s)
                c_src = atb if idx == 1 else cb
                ps_r = [ps_r_pool.tile([128, 512], f32, tag="r",
                                       name=f"ps_r_{idx}_{i}")
                        for i in range(4)]
                lhsT_of_j = lambda j: sT[:, 128 * (j % 4) + 64 * (j // 4):
                                         128 * (j % 4) + 64 * (j // 4) + 64]
                mm2_bank(lhsT_of_j, ps_r[0], 0)
                dve_chain(0, c_src, prep)
                mm2_bank(lhsT_of_j, ps_r[1], 1)
                dve_chain(1, c_src, prep)
                mm2_bank(lhsT_of_j, ps_r[2], 2)
                dve_chain(2, c_src, prep)
                if prep:
                    tr_group(0)
                mm2_bank(lhsT_of_j, ps_r[3], 3)
                dve_chain(3, c_src, prep)
                if prep:
                    tr_group(1)
                    for nb in range(4):
                        sl = slice(512 * nb, 512 * (nb + 1))
                        nc.vector.tensor_tensor(cb[:, sl], gb[:, sl],
                                                cl[:, sl], op=OP.subtract)
                have_pending = prep

            # ---- epilogue: z = soft(q) = q - clamp(q), decoded = z @ wu ----
            nc.vector.tensor_scalar(cl[:], q[:], -LAMBD, LAMBD,
                                    op0=OP.max, op1=OP.min)
            nc.vector.tensor_tensor(t[:], q[:], cl[:], op=OP.subtract)
            nc.sync.dma_start(out=z_d[:, 0:2048], in_=t[0:64, :])
            nc.sync.dma_start(out=z_d[:, 2048:4096], in_=t[64:128, :])

            # decoded = (z @ U1) @ G  (G fp16 streamed from HBM, 2 MB)
            transpose_in(t, pT)
            ps_y = ps_s_pool.tile([128, 512], f32, tag="s", name="ps_y")
            mm1_part(ps_y, 0, 32)
            s_stage(ps_y)
            ps_dec = ps_r_pool.tile([128, 512], f32, tag="r", name="ps_dec")
            for j in range(8):
                stage = wustage.tile([128, M], f16, tag="g", name=f"gst{j}")
                nc.sync.dma_start(out=stage[:], in_=g_d[j])
                lhsT = sT[:, 128 * (j % 4) + 64 * (j // 4):
                          128 * (j % 4) + 64 * (j // 4) + 64]
                nc.tensor.matmul(ps_dec[0:64, :], lhsT, stage[:, 0:512],
                                 start=(j == 0), stop=(j == 7),
                                 tile_position=(0, 0), skip_group_check=True)
                nc.tensor.matmul(ps_dec[64:128, :], lhsT, stage[:, 512:1024],
                                 start=(j == 0), stop=(j == 7),
                                 tile_position=(0, 64), skip_group_check=True)
            dec32 = small.tile([128, 512], f32)
            nc.vector.tensor_copy(dec32[:], ps_dec[:])
            nc.sync.dma_start(out=dec_d[:, 0:512], in_=dec32[0:64, :])
            nc.sync.dma_start(out=dec_d[:, 512:1024], in_=dec32[64:128, :])

    nc.compile()
    return nc


def _host_prep(x, weight):
    """Host-side weight preparation (float64 for the small inversion)."""
    x_flat = np.asarray(x, dtype=np.float32).reshape(B, -1)
    w = np.asarray(weight, dtype=np.float64)
    wu = w / np.linalg.norm(w, axis=1, keepdims=True)        # (4096, 1024)
    G = np.eye(M) + wu.T @ wu
    Gi = np.linalg.inv(G)
    U1 = (wu @ Gi).astype(np.float32)                        # (4096, 1024)
    V2 = np.ascontiguousarray(wu.T).astype(np.float32)       # (1024, 4096)

    u1_16 = U1.astype(np.float16).reshape(32, 128, M)
    v2_16 = V2.astype(np.float16).reshape(8, 128, N)
    g_16 = G.astype(np.float16).reshape(8, 128, M)
    ident = np.eye(128, dtype=np.float32)
    return x_flat, u1_16, v2_16, g_16, ident


def _in_maps(x, weight):
    x_flat, u1_16, v2_16, g_16, ident = _host_prep(x, weight)
    in_maps = []
    for c in range(NCORES):
        rows = x_flat[c * BL:(c + 1) * BL]                   # (64, 1024)
        xT16 = np.ascontiguousarray(rows.T).astype(np.float16).reshape(
            8, 128, BL)
        in_maps.append({
            "xT16": xT16,
            "u1": u1_16,
            "v2": v2_16,
            "g16": g_16,
            "ident": ident,
        })
    return in_maps


def kernel(x, weight):
    from concourse.bass_utils import run_bass_kernel_spmd

    if "nc" not in _PROGRAM_CACHE:
        _PROGRAM_CACHE["nc"] = _build_program(ITERS)
    nc = _PROGRAM_CACHE["nc"]

    in_maps = _in_maps(x, weight)
    res = run_bass_kernel_spmd(nc, in_maps, list(range(NCORES)))
    z = np.concatenate([res.results[c]["z_out"] for c in range(NCORES)], axis=0)
    dec = np.concatenate([res.results[c]["dec_out"] for c in range(NCORES)],
                         axis=0)
    input_shape = np.asarray(x).shape
    return z.astype(np.float32), dec.astype(np.float32).reshape(input_shape)
